# revision 8
# baseline (speedup 1.0000x reference)
"""CreditRiskGNN (2-layer GCN) Trainium2 kernel, 8 NeuronCores — v2.

Sharding: nodes sharded across 8 cores; edges partitioned by destination so
scatter-adds are core-local; per-shard node features all-gathered per layer.

Math: GCNConv(x, W, b)[d] = dinv[d] * (sum_{e: dst=d} h'[src_e] + h'[d]) + b
with h' = dinv (.) (x @ W), dinv = rsqrt(indeg+1).  dinv is folded into x on
the host (h' = (dinv (.) x) @ W1), so phase A is a plain matmul.

v2 structure (one SPMD NEFF):
  A) h' per *piece* (shard split into 4 row-pieces), each piece AllGathered
     as soon as computed -> 4 pipelined AGs into 4 table regions in DRAM.
  B) Aggregation per layer is region(window)-major: pass q only needs AG
     piece q, so collectives hide behind gather/compute of earlier passes.
     Per-(tile, pass) partial sums accumulate in PSUM (bank-granular tiles,
     3 tags x 2 bufs) and drain into an SBUF accumulator per pass.
  C) Gather calls are packed per (super-tile x region) up to the 1024-index
     SWDGE cap (fewer ucode calls => less fixed Q7 overhead). Chunks that
     straddle tile boundaries are handled by partition-sliced matmul segments.
  D) One-hot dst-selection built per 128-chunk with DVE tensor_scalar
     is_equal (per-partition scalar => fast DVE mode, no broadcast penalty).
  E) Layer-1 epilogue per tile runs inside the last pass; r' pieces
     AllGather as soon as their tiles finish, so layer 2's table is ready
     the moment layer 1 ends.

Host does graph preprocessing only (edge sort/pad, gather-index layout,
node-relabeling for the table regions) and the final shard concat.
"""

import contextlib
import ctypes
import math
import os
import sys
import types

import ml_dtypes
import numpy as np

N_CORES = 8
P = 128
D_HID = 64
SUPER = 16                 # tiles per gather super-group
MAX_IDX = 1024             # HW descriptor-ring limit per dma_gather call
MAXNCH = MAX_IDX // P      # chunks per call

LAST_RESULTS = None  # BassKernelResults of the last run (for test harnesses)


# ---------------------------------------------------------------------------
# axon NTFF profile hook shim (only needed when BASS_TRACE=1 under axon)
def _install_axon_profile_shim():
    if "antenv.axon_hooks" in sys.modules:
        return
    try:
        so_path = "/opt/axon/libaxon_pjrt.so"
        if not os.path.exists(so_path):
            return
        lib = ctypes.CDLL(so_path)
        if not hasattr(lib, "axon_start_nrt_profile"):
            return
        lib.axon_start_nrt_profile.argtypes = [
            ctypes.POINTER(ctypes.c_int64),
            ctypes.c_size_t,
        ]
        lib.axon_start_nrt_profile.restype = ctypes.c_int64
        lib.axon_stop_nrt_profile.argtypes = [ctypes.c_char_p]
        lib.axon_stop_nrt_profile.restype = ctypes.c_int64

        @contextlib.contextmanager
        def _hook(output_dir, device_ids):
            import jax

            jax.devices()
            if device_ids:
                ids = (ctypes.c_int64 * len(device_ids))(*device_ids)
                rc = lib.axon_start_nrt_profile(ids, len(device_ids))
            else:
                rc = lib.axon_start_nrt_profile(None, 0)
            if rc != 0:
                raise RuntimeError(f"axon_start_nrt_profile rc={rc}")
            try:
                yield
            finally:
                n = lib.axon_stop_nrt_profile(str(output_dir).encode())
                if n < 0:
                    raise RuntimeError(f"axon_stop_nrt_profile rc={n}")

        mod = types.ModuleType("antenv.axon_hooks")
        _state = {"hook": _hook}
        mod.set_axon_ntff_profile_hook = lambda h: _state.__setitem__("hook", h)
        mod.get_axon_ntff_profile_hook = lambda: _state["hook"]
        sys.modules["antenv.axon_hooks"] = mod
        import antenv

        antenv.axon_hooks = mod
    except Exception:
        pass


# ---------------------------------------------------------------------------
# Host-side graph preprocessing


def _build_plan(src, dst, n_nodes, n_cores):
    """Shared (cross-core-uniform) program structure + per-core gather data.

    Table layout: 4 regions; region q holds rows [c*pieces[q] + r] for shard
    row r in piece q of core c (so AllGather piece q fills region q exactly).
    """
    sh = n_nodes // n_cores
    n_tiles = math.ceil(sh / P)
    piece = math.ceil(sh / 4 / P) * P
    pieces = [piece, piece, piece, sh - 3 * piece]
    assert pieces[3] > 0
    piece_starts = np.array([0, piece, 2 * piece, 3 * piece], dtype=np.int64)
    regions = [n_cores * pc for pc in pieces]
    n_q = 4
    n_sup = math.ceil(n_tiles / SUPER)
    sup_tiles = [
        list(range(s * SUPER, min((s + 1) * SUPER, n_tiles))) for s in range(n_sup)
    ]

    # --- per-core edge partition, sorted by (tile, region, table-idx)
    core_of = dst // sh
    counts = np.zeros((n_cores, n_tiles, n_q), dtype=np.int64)
    per_core_sorted = []
    for c in range(n_cores):
        m = core_of == c
        s_c = src[m].astype(np.int64)
        d_c = (dst[m] - c * sh).astype(np.int64)
        c_s, r_s = np.divmod(s_c, sh)
        q_s = np.searchsorted(piece_starts, r_s, side="right") - 1
        tab = c_s * np.array(pieces)[q_s] + (r_s - piece_starts[q_s])
        t_of = d_c // P
        key = t_of * n_q + q_s
        order = np.lexsort((tab, key))
        s_key = key[order]
        tab_s, d_s = tab[order], d_c[order]
        allk = np.arange(n_tiles * n_q)
        starts = np.searchsorted(s_key, allk, side="left").reshape(n_tiles, n_q)
        ends = np.searchsorted(s_key, allk, side="right").reshape(n_tiles, n_q)
        counts[c] = ends - starts
        per_core_sorted.append((tab_s, d_s, starts))

    # pad each (tile, region) run to a multiple of 128 so gather chunks never
    # straddle tiles (PE matmul operands must start at partition 0)
    mx = counts.max(axis=0)
    padded = ((mx + P - 1) // P) * P  # [n_tiles, n_q]; 0 stays 0

    # first pass (q) with edges, per tile — drain uses copy there, add after
    first_q = np.full(n_tiles, -1, dtype=np.int64)
    for t in range(n_tiles):
        nz = np.nonzero(padded[t])[0]
        if len(nz):
            first_q[t] = nz[0]

    # --- processing order: (q, s) groups; calls packed to MAX_IDX
    calls = []  # dict: q, o16, ni, nch, chunk0, segs
    slot_t_parts, slot_q_parts, slot_rank_parts = [], [], []
    call_starts = []
    total_idx = 0
    total_chunks = 0
    program = []  # ('call', ci) | ('drain', q, s) | ('epi', s)

    for q in range(n_q):
        for s in range(n_sup):
            runs = [(t, int(padded[t, q])) for t in sup_tiles[s] if padded[t, q] > 0]
            ni_group = sum(n for _, n in runs)
            if ni_group == 0:
                if q == n_q - 1:
                    program.append(("epi", s))
                continue
            g_t = np.concatenate([np.full(n, t, dtype=np.int64) for t, n in runs])
            g_rank = np.concatenate([np.arange(n, dtype=np.int64) for _, n in runs])
            slot_t_parts.append(g_t)
            slot_q_parts.append(np.full(ni_group, q, dtype=np.int64))
            slot_rank_parts.append(g_rank)
            # per-tile first/last segment flags within this group
            seen_first = set()
            last_seg_of_t = {}
            done = 0
            while done < ni_group:
                take = min(MAX_IDX, ni_group - done)
                nch = (take + P - 1) // P
                ci = len(calls)
                call_starts.append(total_idx + done)
                segs = []
                ct = g_t[done : done + take]
                for k in range(nch):
                    a = k * P
                    b = min((k + 1) * P, take)
                    kt = ct[a:b]
                    cuts = [0] + list(np.nonzero(np.diff(kt))[0] + 1) + [b - a]
                    for j in range(len(cuts) - 1):
                        aa, bb = cuts[j], cuts[j + 1]
                        t = int(kt[aa])
                        st_f = t not in seen_first
                        seen_first.add(t)
                        seg = [k, aa, bb, t, st_f, False]
                        last_seg_of_t[t] = (ci, len(segs))
                        segs.append(seg)
                calls.append(
                    dict(q=q, o16=(total_idx + done) // 16, ni=take, nch=nch,
                         chunk0=total_chunks, segs=segs)
                )
                total_chunks += nch
                program.append(("call", ci))
                done += take
            for t, (ci, si) in last_seg_of_t.items():
                calls[ci]["segs"][si][5] = True
            if q == n_q - 1:
                program.append(("epi", s))
            total_idx += ni_group

    slot_t = np.concatenate(slot_t_parts)
    slot_q = np.concatenate(slot_q_parts)
    slot_rank = np.concatenate(slot_rank_parts)
    call_starts_a = np.array(call_starts, dtype=np.int64)

    # global slot -> (chunk col, chunk row) for dstloc
    i_all = np.arange(total_idx, dtype=np.int64)
    ci_of = np.searchsorted(call_starts_a, i_all, side="right") - 1
    rel = i_all - call_starts_a[ci_of]
    chunk0_of = np.array([c["chunk0"] for c in calls], dtype=np.int64)[ci_of]
    col_of = chunk0_of + rel // P
    row_of = rel % P

    meta = dict(
        n_nodes=n_nodes,
        sh=sh,
        n_tiles=n_tiles,
        pieces=pieces,
        regions=regions,
        n_q=n_q,
        sup_tiles=sup_tiles,
        calls=calls,
        program=program,
        total_idx=total_idx,
        total_chunks=total_chunks,
        padded=padded,
        first_q=first_q,
    )

    # --- per-core gather index / dstloc data
    per_core = []
    for c in range(n_cores):
        tab_s, d_s, starts = per_core_sorted[c]
        cnt = counts[c][slot_t, slot_q]
        st = starts[slot_t, slot_q]
        valid = slot_rank < cnt
        if len(tab_s):
            pos = np.clip(st + slot_rank, 0, len(tab_s) - 1)
            idxv = np.where(valid, tab_s[pos], 0).astype(np.int16)
            dstv = np.where(valid, d_s[pos] % P, -1).astype(np.float32)
        else:
            idxv = np.zeros(total_idx, dtype=np.int16)
            dstv = np.full(total_idx, -1.0, dtype=np.float32)
        arr16 = np.zeros((16, total_idx // 16), dtype=np.int16)
        arr16[i_all % 16, i_all // 16] = idxv
        idx_arr = np.tile(arr16, (8, 1))
        dst_arr = np.full((P, total_chunks), -1.0, dtype=np.float32)
        dst_arr[row_of, col_of] = dstv
        per_core.append((idx_arr, dst_arr))
    return meta, per_core


# ---------------------------------------------------------------------------
# Device program


def _build_program(meta):
    import concourse.bacc as bacc
    import concourse.mybir as mybir
    import concourse.tile as tile

    sh = meta["sh"]
    n_tiles = meta["n_tiles"]
    pieces = meta["pieces"]
    regions = meta["regions"]
    n_q = meta["n_q"]
    sup_tiles = meta["sup_tiles"]
    calls = meta["calls"]
    program = meta["program"]
    total_idx = meta["total_idx"]
    total_chunks = meta["total_chunks"]
    padded = meta["padded"]
    first_q = meta["first_q"]

    piece_tiles = [(pc + P - 1) // P for pc in pieces]
    # tile -> (piece, row offset within piece)
    tile_piece = []
    for t in range(n_tiles):
        row = t * P
        pacc = 0
        for p in range(n_q):
            if row < pacc + pieces[p]:
                tile_piece.append((p, row - pacc))
                break
            pacc += pieces[p]
    # piece p fully epilogued once tiles < cum_tiles[p] are done
    cum_tiles = np.cumsum(piece_tiles)

    f32 = mybir.dt.float32
    bf16 = mybir.dt.bfloat16
    nc = bacc.Bacc("TRN2", target_bir_lowering=False, debug=False, num_swdge_queues=4)

    xT = nc.dram_tensor("xT", [P, sh], f32, kind="ExternalInput")
    w1 = nc.dram_tensor("w1", [P, D_HID], f32, kind="ExternalInput")
    b1bc = nc.dram_tensor("b1bc", [P, D_HID], f32, kind="ExternalInput")
    w2bc = nc.dram_tensor("w2bc", [P, D_HID], f32, kind="ExternalInput")
    dinv_sh = nc.dram_tensor("dinv_sh", [P, n_tiles], f32, kind="ExternalInput")
    iotab = nc.dram_tensor("iotab", [P, P], bf16, kind="ExternalInput")
    idx16 = nc.dram_tensor(
        "idx16", [P, total_idx // 16], mybir.dt.int16, kind="ExternalInput"
    )
    dstloc = nc.dram_tensor("dstloc", [P, total_chunks], f32, kind="ExternalInput")
    b2col = nc.dram_tensor("b2col", [P, 1], f32, kind="ExternalInput")
    y_out = nc.dram_tensor("y", [sh, 1], f32, kind="ExternalOutput")

    h_shp = [
        nc.dram_tensor(f"h_sh{p}", [pieces[p], D_HID], f32, kind="Internal")
        for p in range(n_q)
    ]
    h_r = [
        nc.dram_tensor(
            f"h_r{p}", [regions[p], D_HID], f32, kind="Internal", addr_space="Shared"
        )
        for p in range(n_q)
    ]
    r_shp = [
        nc.dram_tensor(f"r_sh{p}", [pieces[p], D_HID], f32, kind="Internal")
        for p in range(n_q)
    ]
    r_r = [
        nc.dram_tensor(
            f"r_r{p}", [regions[p], D_HID], f32, kind="Internal", addr_space="Shared"
        )
        for p in range(n_q)
    ]

    rg = [list(range(N_CORES))]

    with tile.TileContext(nc) as tc:
        with (
            tc.tile_pool(name="const", bufs=1) as cpool,
            tc.tile_pool(name="sbuf", bufs=1) as pool,
            tc.tile_pool(name="psum", bufs=1, space="PSUM") as psum_pool,
        ):
            w1_t = cpool.tile([P, D_HID], f32)
            nc.sync.dma_start(w1_t[:], w1[:])
            b1_t = cpool.tile([P, D_HID], f32)
            nc.sync.dma_start(b1_t[:], b1bc[:])
            w2_t = cpool.tile([P, D_HID], f32)
            nc.sync.dma_start(w2_t[:], w2bc[:])
            dinv_t = cpool.tile([P, n_tiles], f32)
            nc.sync.dma_start(dinv_t[:], dinv_sh[:])
            iota_t = cpool.tile([P, P], bf16)
            nc.sync.dma_start(iota_t[:], iotab[:])
            idx_t = cpool.tile([P, total_idx // 16], mybir.dt.int16)
            nc.sync.dma_start(idx_t[:], idx16[:])
            dl_t = cpool.tile([P, total_chunks], f32)
            nc.sync.dma_start(dl_t[:], dstloc[:])
            b2_t = cpool.tile([P, 1], f32)
            nc.sync.dma_start(b2_t[:], b2col[:])
            acc = cpool.tile([P, n_tiles, D_HID], f32)

            # ---- phase A: h' = xs @ W1 per piece; AG piece when stored
            B4 = 4
            for pnum in range(n_q):
                t0 = int(sum(piece_tiles[:pnum]))
                t1 = t0 + piece_tiles[pnum]
                for t4 in range(t0, t1, B4):
                    nb = min(B4, t1 - t4)
                    c0 = t4 * P
                    cn = min(sh, (t4 + nb) * P) - c0
                    xt = pool.tile([P, B4 * P], f32, tag="xt", bufs=3)
                    nc.sync.dma_start(xt[:, :cn], xT[:, c0 : c0 + cn])
                    hs4 = pool.tile([P, B4, D_HID], f32, tag="hs", bufs=3)
                    for j in range(nb):
                        t = t4 + j
                        pt = min(P, sh - t * P)
                        ph = psum_pool.tile(
                            [P, D_HID], f32, tag="ph", bufs=2, space="PSUM"
                        )
                        nc.tensor.matmul(
                            ph[:pt, :],
                            lhsT=xt[:, j * P : j * P + pt],
                            rhs=w1_t[:],
                            start=True,
                            stop=True,
                        )
                        nc.vector.tensor_copy(out=hs4[:pt, j, :], in_=ph[:pt, :])
                    prow = c0 - int(sum(pieces[:pnum]))
                    if cn == nb * P:
                        dst_ap = h_shp[pnum][prow : prow + cn, :].rearrange(
                            "(j p) d -> p j d", p=P
                        )
                        nc.sync.dma_start(dst_ap, hs4[:, :nb, :])
                    else:
                        for j in range(nb):
                            t = t4 + j
                            pt = min(P, sh - t * P)
                            pr = prow + j * P
                            nc.sync.dma_start(
                                h_shp[pnum][pr : pr + pt, :], hs4[:pt, j, :]
                            )
                nc.gpsimd.collective_compute(
                    "AllGather",
                    mybir.AluOpType.bypass,
                    replica_groups=rg,
                    ins=[h_shp[pnum][:]],
                    outs=[h_r[pnum][:]],
                )

            # ---- aggregation layers
            qn_state = [0]

            def do_layer(layer, table_r, self_p, out_p):
                pa_cur = {}  # t -> live psum tile for current (t, q) chain
                ag_fired = [False] * n_q

                def epilogue(s):
                    for t in sup_tiles[s]:
                        pt = min(P, sh - t * P)
                        pnum, prow = tile_piece[t]
                        st = pool.tile([P, D_HID], f32, tag=f"st{layer}", bufs=3)
                        if pt < P:
                            nc.vector.memset(st[:], 0.0)
                        nc.sync.dma_start(
                            st[:pt, :], self_p[pnum][prow : prow + pt, :]
                        )
                        dv = dinv_t[:pt, t : t + 1]
                        if first_q[t] >= 0:
                            u1 = pool.tile([P, D_HID], f32, tag=f"u1{layer}", bufs=3)
                            nc.vector.tensor_add(
                                u1[:pt, :], acc[:pt, t, :], st[:pt, :]
                            )
                        else:
                            u1 = st
                        if layer == 1:
                            t2 = pool.tile([P, D_HID], f32, tag="t2", bufs=3)
                            nc.vector.tensor_scalar_mul(t2[:pt, :], u1[:pt, :], dv)
                            t3 = pool.tile([P, D_HID], f32, tag="t3", bufs=3)
                            nc.vector.tensor_add(t3[:pt, :], t2[:pt, :], b1_t[:pt, :])
                            rr = pool.tile([P, D_HID], f32, tag="rr", bufs=3)
                            nc.scalar.activation(
                                rr[:pt, :],
                                t3[:pt, :],
                                mybir.ActivationFunctionType.Relu,
                            )
                            rp = pool.tile([P, D_HID], f32, tag="rp", bufs=3)
                            nc.vector.tensor_scalar_mul(rp[:pt, :], rr[:pt, :], dv)
                            nc.sync.dma_start(
                                out_p[pnum][prow : prow + pt, :], rp[:pt, :]
                            )
                        else:
                            u2 = pool.tile([P, D_HID], f32, tag="u2", bufs=3)
                            nc.vector.tensor_mul(u2[:pt, :], u1[:pt, :], w2_t[:pt, :])
                            yv = pool.tile([P, 1], f32, tag="yv", bufs=3)
                            nc.vector.tensor_reduce(
                                yv[:pt, :],
                                u2[:pt, :],
                                axis=mybir.AxisListType.X,
                                op=mybir.AluOpType.add,
                            )
                            ov = pool.tile([P, 1], f32, tag="ov", bufs=3)
                            nc.scalar.activation(
                                ov[:pt, :],
                                yv[:pt, :],
                                mybir.ActivationFunctionType.Sigmoid,
                                bias=b2_t[:pt, :],
                                scale=dv,
                            )
                            nc.sync.dma_start(y_out[t * P : t * P + pt, :], ov[:pt, :])

                for item in program:
                    if item[0] == "call":
                        c = calls[item[1]]
                        q, ni, nch = c["q"], c["ni"], c["nch"]
                        gbuf = pool.tile(
                            [P, MAXNCH, D_HID], f32, tag=f"g{layer}", bufs=3
                        )
                        nc.gpsimd.dma_gather(
                            gbuf[:, :nch, :],
                            table_r[q][0 : regions[q], :],
                            idx_t[:, c["o16"] : c["o16"] + ni // 16],
                            ni,
                            ni,
                            D_HID,
                            queue_num=qn_state[0] % 4,
                        )
                        qn_state[0] += 1
                        gbf = pool.tile(
                            [P, MAXNCH, D_HID], bf16, tag=f"gb{layer}", bufs=3
                        )
                        nc.scalar.copy(out=gbf[:, :nch, :], in_=gbuf[:, :nch, :])
                        oh = pool.tile([P, MAXNCH, P], bf16, tag=f"oh{layer}", bufs=3)
                        for k in range(nch):
                            ch = c["chunk0"] + k
                            nc.vector.tensor_scalar(
                                oh[:, k, :],
                                iota_t[:],
                                dl_t[:, ch : ch + 1],
                                None,
                                op0=mybir.AluOpType.is_equal,
                            )
                        for (k, a, b, t, st_f, sp_f) in c["segs"]:
                            if st_f:
                                pa_cur[t] = psum_pool.tile(
                                    [P, D_HID],
                                    f32,
                                    name=f"pa{t % 3}",
                                    tag=f"pa{t % 3}",
                                    bufs=2,
                                    space="PSUM",
                                )
                            nc.tensor.matmul(
                                pa_cur[t][:],
                                lhsT=oh[a:b, k, :],
                                rhs=gbf[a:b, k, :],
                                start=st_f,
                                stop=sp_f,
                            )
                            if sp_f:
                                # chain done: drain PSUM into the SBUF acc
                                if q == first_q[t]:
                                    nc.vector.tensor_copy(
                                        out=acc[:, t, :], in_=pa_cur[t][:]
                                    )
                                else:
                                    nc.vector.tensor_add(
                                        acc[:, t, :], acc[:, t, :], pa_cur[t][:]
                                    )
                    else:
                        _, s = item
                        epilogue(s)
                        if layer == 1:
                            done_t = sup_tiles[s][-1] + 1
                            for pnum in range(n_q):
                                if not ag_fired[pnum] and done_t >= cum_tiles[pnum]:
                                    nc.gpsimd.collective_compute(
                                        "AllGather",
                                        mybir.AluOpType.bypass,
                                        replica_groups=rg,
                                        ins=[out_p[pnum][:]],
                                        outs=[r_r[pnum][:]],
                                    )
                                    ag_fired[pnum] = True

            do_layer(1, h_r, h_shp, r_shp)
            do_layer(2, r_r, r_shp, None)

    nc.compile()
    return nc


# ---------------------------------------------------------------------------


def kernel(**inputs) -> np.ndarray:
    global LAST_RESULTS
    x = np.asarray(inputs["x"], dtype=np.float32)
    edge_index = np.asarray(inputs["edge_index"])
    w1_in = np.asarray(inputs["W1"], dtype=np.float32)
    b1_in = np.asarray(inputs["b1"], dtype=np.float32)
    w2_in = np.asarray(inputs["W2"], dtype=np.float32)
    b2_in = np.asarray(inputs["b2"], dtype=np.float32)

    n_nodes = x.shape[0]
    src = edge_index[0].astype(np.int64)
    dst = edge_index[1].astype(np.int64)

    deg = np.bincount(dst, minlength=n_nodes).astype(np.float64) + 1.0
    dinv = (1.0 / np.sqrt(deg)).astype(np.float32)

    meta, per_core = _build_plan(src, dst, n_nodes, N_CORES)
    sh = meta["sh"]
    n_tiles = meta["n_tiles"]

    nc = _build_program(meta)

    iota_arr = (
        np.broadcast_to(np.arange(P, dtype=np.float32), (P, P))
        .astype(ml_dtypes.bfloat16)
        .copy()
    )
    b1bc = np.broadcast_to(b1_in.reshape(1, D_HID), (P, D_HID)).copy()
    w2bc = np.broadcast_to(w2_in.reshape(1, D_HID), (P, D_HID)).copy()

    xs = x * dinv[:, None]  # fold dinv into x (h' = (dinv.x) @ W1)

    in_maps = []
    for c in range(N_CORES):
        idx_arr, dst_arr = per_core[c]
        xT = np.ascontiguousarray(xs[c * sh : (c + 1) * sh].T)  # [128, sh]
        dv = np.zeros((P, n_tiles), dtype=np.float32)
        dsl = dinv[c * sh : (c + 1) * sh]
        for t in range(n_tiles):
            pt = min(P, sh - t * P)
            dv[:pt, t] = dsl[t * P : t * P + pt]
        in_maps.append(
            {
                "xT": xT,
                "w1": w1_in,
                "b1bc": b1bc,
                "w2bc": w2bc,
                "dinv_sh": dv,
                "iotab": iota_arr,
                "idx16": idx_arr,
                "dstloc": dst_arr,
                "b2col": np.full((P, 1), float(b2_in.reshape(-1)[0]), dtype=np.float32),
            }
        )

    from concourse import bass_utils

    if os.environ.get("BASS_TRACE"):
        _install_axon_profile_shim()

    res = bass_utils.run_bass_kernel_spmd(
        nc,
        in_maps,
        core_ids=list(range(N_CORES)),
        trace=bool(os.environ.get("BASS_TRACE")),
        trace_cores=[0] if os.environ.get("BASS_TRACE") else None,
    )
    LAST_RESULTS = res
    out = np.concatenate([res.results[c]["y"] for c in range(N_CORES)], axis=0)
    return out.astype(np.float32)


# revision 9
# speedup vs baseline: 1.0232x; 1.0232x over previous
"""CreditRiskGNN (2-layer GCN) Trainium2 kernel, 8 NeuronCores — v2.

Sharding: nodes sharded across 8 cores; edges partitioned by destination so
scatter-adds are core-local; per-shard node features all-gathered per layer.

Math: GCNConv(x, W, b)[d] = dinv[d] * (sum_{e: dst=d} h'[src_e] + h'[d]) + b
with h' = dinv (.) (x @ W), dinv = rsqrt(indeg+1).  dinv is folded into x on
the host (h' = (dinv (.) x) @ W1), so phase A is a plain matmul.

v2 structure (one SPMD NEFF):
  A) h' per *piece* (shard split into 4 row-pieces), each piece AllGathered
     as soon as computed -> 4 pipelined AGs into 4 table regions in DRAM.
  B) Aggregation per layer is region(window)-major: pass q only needs AG
     piece q, so collectives hide behind gather/compute of earlier passes.
     Per-(tile, pass) partial sums accumulate in PSUM (bank-granular tiles,
     3 tags x 2 bufs) and drain into an SBUF accumulator per pass.
  C) Gather calls are packed per (super-tile x region) up to the 1024-index
     SWDGE cap (fewer ucode calls => less fixed Q7 overhead). Chunks that
     straddle tile boundaries are handled by partition-sliced matmul segments.
  D) One-hot dst-selection built per 128-chunk with DVE tensor_scalar
     is_equal (per-partition scalar => fast DVE mode, no broadcast penalty).
  E) Layer-1 epilogue per tile runs inside the last pass; r' pieces
     AllGather as soon as their tiles finish, so layer 2's table is ready
     the moment layer 1 ends.

Host does graph preprocessing only (edge sort/pad, gather-index layout,
node-relabeling for the table regions) and the final shard concat.
"""

import contextlib
import ctypes
import math
import os
import sys
import types

import ml_dtypes
import numpy as np

N_CORES = 8
P = 128
D_HID = 64
SUPER = 16                 # tiles per gather super-group
MAX_IDX = 1024             # HW descriptor-ring limit per dma_gather call
MAXNCH = MAX_IDX // P      # chunks per call

LAST_RESULTS = None  # BassKernelResults of the last run (for test harnesses)


# ---------------------------------------------------------------------------
# axon NTFF profile hook shim (only needed when BASS_TRACE=1 under axon)
def _install_axon_profile_shim():
    if "antenv.axon_hooks" in sys.modules:
        return
    try:
        so_path = "/opt/axon/libaxon_pjrt.so"
        if not os.path.exists(so_path):
            return
        lib = ctypes.CDLL(so_path)
        if not hasattr(lib, "axon_start_nrt_profile"):
            return
        lib.axon_start_nrt_profile.argtypes = [
            ctypes.POINTER(ctypes.c_int64),
            ctypes.c_size_t,
        ]
        lib.axon_start_nrt_profile.restype = ctypes.c_int64
        lib.axon_stop_nrt_profile.argtypes = [ctypes.c_char_p]
        lib.axon_stop_nrt_profile.restype = ctypes.c_int64

        @contextlib.contextmanager
        def _hook(output_dir, device_ids):
            import jax

            jax.devices()
            if device_ids:
                ids = (ctypes.c_int64 * len(device_ids))(*device_ids)
                rc = lib.axon_start_nrt_profile(ids, len(device_ids))
            else:
                rc = lib.axon_start_nrt_profile(None, 0)
            if rc != 0:
                raise RuntimeError(f"axon_start_nrt_profile rc={rc}")
            try:
                yield
            finally:
                n = lib.axon_stop_nrt_profile(str(output_dir).encode())
                if n < 0:
                    raise RuntimeError(f"axon_stop_nrt_profile rc={n}")

        mod = types.ModuleType("antenv.axon_hooks")
        _state = {"hook": _hook}
        mod.set_axon_ntff_profile_hook = lambda h: _state.__setitem__("hook", h)
        mod.get_axon_ntff_profile_hook = lambda: _state["hook"]
        sys.modules["antenv.axon_hooks"] = mod
        import antenv

        antenv.axon_hooks = mod
    except Exception:
        pass


# ---------------------------------------------------------------------------
# Host-side graph preprocessing


def _build_plan(src, dst, n_nodes, n_cores):
    """Shared (cross-core-uniform) program structure + per-core gather data.

    Table layout: 4 regions; region q holds rows [c*pieces[q] + r] for shard
    row r in piece q of core c (so AllGather piece q fills region q exactly).
    """
    sh = n_nodes // n_cores
    n_tiles = math.ceil(sh / P)
    piece = math.ceil(sh / 4 / P) * P
    pieces = [piece, piece, piece, sh - 3 * piece]
    assert pieces[3] > 0
    piece_starts = np.array([0, piece, 2 * piece, 3 * piece], dtype=np.int64)
    regions = [n_cores * pc for pc in pieces]
    n_q = 4
    n_sup = math.ceil(n_tiles / SUPER)
    sup_tiles = [
        list(range(s * SUPER, min((s + 1) * SUPER, n_tiles))) for s in range(n_sup)
    ]

    # --- per-core edge partition, sorted by (tile, region, table-idx)
    core_of = dst // sh
    counts = np.zeros((n_cores, n_tiles, n_q), dtype=np.int64)
    per_core_sorted = []
    for c in range(n_cores):
        m = core_of == c
        s_c = src[m].astype(np.int64)
        d_c = (dst[m] - c * sh).astype(np.int64)
        c_s, r_s = np.divmod(s_c, sh)
        q_s = np.searchsorted(piece_starts, r_s, side="right") - 1
        tab = c_s * np.array(pieces)[q_s] + (r_s - piece_starts[q_s])
        t_of = d_c // P
        key = t_of * n_q + q_s
        order = np.lexsort((tab, key))
        s_key = key[order]
        tab_s, d_s = tab[order], d_c[order]
        allk = np.arange(n_tiles * n_q)
        starts = np.searchsorted(s_key, allk, side="left").reshape(n_tiles, n_q)
        ends = np.searchsorted(s_key, allk, side="right").reshape(n_tiles, n_q)
        counts[c] = ends - starts
        per_core_sorted.append((tab_s, d_s, starts))

    # pad each (tile, region) run to a multiple of 128 so gather chunks never
    # straddle tiles (PE matmul operands must start at partition 0)
    mx = counts.max(axis=0)
    padded = ((mx + P - 1) // P) * P  # [n_tiles, n_q]; 0 stays 0

    # first pass (q) with edges, per tile — drain uses copy there, add after
    first_q = np.full(n_tiles, -1, dtype=np.int64)
    for t in range(n_tiles):
        nz = np.nonzero(padded[t])[0]
        if len(nz):
            first_q[t] = nz[0]

    # --- processing order: (q, s) groups; calls packed to MAX_IDX
    calls = []  # dict: q, o16, ni, nch, chunk0, segs
    slot_t_parts, slot_q_parts, slot_rank_parts = [], [], []
    call_starts = []
    total_idx = 0
    total_chunks = 0
    program = []  # ('call', ci) | ('drain', q, s) | ('epi', s)

    for q in range(n_q):
        for s in range(n_sup):
            runs = [(t, int(padded[t, q])) for t in sup_tiles[s] if padded[t, q] > 0]
            ni_group = sum(n for _, n in runs)
            if ni_group == 0:
                if q == n_q - 1:
                    program.append(("epi", s))
                continue
            g_t = np.concatenate([np.full(n, t, dtype=np.int64) for t, n in runs])
            g_rank = np.concatenate([np.arange(n, dtype=np.int64) for _, n in runs])
            slot_t_parts.append(g_t)
            slot_q_parts.append(np.full(ni_group, q, dtype=np.int64))
            slot_rank_parts.append(g_rank)
            # per-tile first/last segment flags within this group
            seen_first = set()
            last_seg_of_t = {}
            done = 0
            while done < ni_group:
                take = min(MAX_IDX, ni_group - done)
                nch = (take + P - 1) // P
                ci = len(calls)
                call_starts.append(total_idx + done)
                segs = []
                ct = g_t[done : done + take]
                for k in range(nch):
                    a = k * P
                    b = min((k + 1) * P, take)
                    kt = ct[a:b]
                    cuts = [0] + list(np.nonzero(np.diff(kt))[0] + 1) + [b - a]
                    for j in range(len(cuts) - 1):
                        aa, bb = cuts[j], cuts[j + 1]
                        t = int(kt[aa])
                        st_f = t not in seen_first
                        seen_first.add(t)
                        seg = [k, aa, bb, t, st_f, False]
                        last_seg_of_t[t] = (ci, len(segs))
                        segs.append(seg)
                calls.append(
                    dict(q=q, o16=(total_idx + done) // 16, ni=take, nch=nch,
                         chunk0=total_chunks, segs=segs)
                )
                total_chunks += nch
                program.append(("call", ci))
                done += take
            for t, (ci, si) in last_seg_of_t.items():
                calls[ci]["segs"][si][5] = True
            if q == n_q - 1:
                program.append(("epi", s))
            total_idx += ni_group

    slot_t = np.concatenate(slot_t_parts)
    slot_q = np.concatenate(slot_q_parts)
    slot_rank = np.concatenate(slot_rank_parts)
    call_starts_a = np.array(call_starts, dtype=np.int64)

    # global slot -> (chunk col, chunk row) for dstloc
    i_all = np.arange(total_idx, dtype=np.int64)
    ci_of = np.searchsorted(call_starts_a, i_all, side="right") - 1
    rel = i_all - call_starts_a[ci_of]
    chunk0_of = np.array([c["chunk0"] for c in calls], dtype=np.int64)[ci_of]
    col_of = chunk0_of + rel // P
    row_of = rel % P

    meta = dict(
        n_nodes=n_nodes,
        sh=sh,
        n_tiles=n_tiles,
        pieces=pieces,
        regions=regions,
        n_q=n_q,
        sup_tiles=sup_tiles,
        calls=calls,
        program=program,
        total_idx=total_idx,
        total_chunks=total_chunks,
        padded=padded,
        first_q=first_q,
    )

    # --- per-core gather index / dstloc data
    per_core = []
    for c in range(n_cores):
        tab_s, d_s, starts = per_core_sorted[c]
        cnt = counts[c][slot_t, slot_q]
        st = starts[slot_t, slot_q]
        valid = slot_rank < cnt
        if len(tab_s):
            pos = np.clip(st + slot_rank, 0, len(tab_s) - 1)
            idxv = np.where(valid, tab_s[pos], 0).astype(np.int16)
            dstv = np.where(valid, d_s[pos] % P, -1).astype(np.float32)
        else:
            idxv = np.zeros(total_idx, dtype=np.int16)
            dstv = np.full(total_idx, -1.0, dtype=np.float32)
        arr16 = np.zeros((16, total_idx // 16), dtype=np.int16)
        arr16[i_all % 16, i_all // 16] = idxv
        idx_arr = np.tile(arr16, (8, 1))
        dst_arr = np.full((P, total_chunks), -1.0, dtype=np.float32)
        dst_arr[row_of, col_of] = dstv
        per_core.append((idx_arr, dst_arr))
    return meta, per_core


# ---------------------------------------------------------------------------
# Device program


def _build_program(meta):
    import concourse.bacc as bacc
    import concourse.mybir as mybir
    import concourse.tile as tile

    sh = meta["sh"]
    n_tiles = meta["n_tiles"]
    pieces = meta["pieces"]
    regions = meta["regions"]
    n_q = meta["n_q"]
    sup_tiles = meta["sup_tiles"]
    calls = meta["calls"]
    program = meta["program"]
    total_idx = meta["total_idx"]
    total_chunks = meta["total_chunks"]
    padded = meta["padded"]
    first_q = meta["first_q"]

    piece_tiles = [(pc + P - 1) // P for pc in pieces]
    # tile -> (piece, row offset within piece)
    tile_piece = []
    for t in range(n_tiles):
        row = t * P
        pacc = 0
        for p in range(n_q):
            if row < pacc + pieces[p]:
                tile_piece.append((p, row - pacc))
                break
            pacc += pieces[p]
    # piece p fully epilogued once tiles < cum_tiles[p] are done
    cum_tiles = np.cumsum(piece_tiles)

    f32 = mybir.dt.float32
    bf16 = mybir.dt.bfloat16
    nc = bacc.Bacc("TRN2", target_bir_lowering=False, debug=False, num_swdge_queues=4)

    xT = nc.dram_tensor("xT", [P, sh], f32, kind="ExternalInput")
    w1 = nc.dram_tensor("w1", [P, D_HID], f32, kind="ExternalInput")
    b1bc = nc.dram_tensor("b1bc", [P, D_HID], f32, kind="ExternalInput")
    w2bc = nc.dram_tensor("w2bc", [P, D_HID], f32, kind="ExternalInput")
    dinv_sh = nc.dram_tensor("dinv_sh", [P, n_tiles], f32, kind="ExternalInput")
    iotab = nc.dram_tensor("iotab", [P, MAXNCH, P], bf16, kind="ExternalInput")
    idx16 = nc.dram_tensor(
        "idx16", [P, total_idx // 16], mybir.dt.int16, kind="ExternalInput"
    )
    dstloc = nc.dram_tensor("dstloc", [P, total_chunks], bf16, kind="ExternalInput")
    b2col = nc.dram_tensor("b2col", [P, 1], f32, kind="ExternalInput")
    y_out = nc.dram_tensor("y", [sh, 1], f32, kind="ExternalOutput")

    h_shp = [
        nc.dram_tensor(f"h_sh{p}", [pieces[p], D_HID], f32, kind="Internal")
        for p in range(n_q)
    ]
    h_r = [
        nc.dram_tensor(
            f"h_r{p}", [regions[p], D_HID], f32, kind="Internal", addr_space="Shared"
        )
        for p in range(n_q)
    ]
    r_shp = [
        nc.dram_tensor(f"r_sh{p}", [pieces[p], D_HID], f32, kind="Internal")
        for p in range(n_q)
    ]
    r_r = [
        nc.dram_tensor(
            f"r_r{p}", [regions[p], D_HID], f32, kind="Internal", addr_space="Shared"
        )
        for p in range(n_q)
    ]

    rg = [list(range(N_CORES))]

    with tile.TileContext(nc) as tc:
        with (
            tc.tile_pool(name="const", bufs=1) as cpool,
            tc.tile_pool(name="sbuf", bufs=1) as pool,
            tc.tile_pool(name="psum", bufs=1, space="PSUM") as psum_pool,
        ):
            w1_t = cpool.tile([P, D_HID], f32)
            nc.sync.dma_start(w1_t[:], w1[:])
            b1_t = cpool.tile([P, D_HID], f32)
            nc.sync.dma_start(b1_t[:], b1bc[:])
            w2_t = cpool.tile([P, D_HID], f32)
            nc.sync.dma_start(w2_t[:], w2bc[:])
            dinv_t = cpool.tile([P, n_tiles], f32)
            nc.sync.dma_start(dinv_t[:], dinv_sh[:])
            iota_t = cpool.tile([P, MAXNCH, P], bf16)
            nc.sync.dma_start(iota_t[:], iotab[:])
            idx_t = cpool.tile([P, total_idx // 16], mybir.dt.int16)
            nc.sync.dma_start(idx_t[:], idx16[:])
            dl_t = cpool.tile([P, total_chunks], bf16)
            nc.sync.dma_start(dl_t[:], dstloc[:])
            b2_t = cpool.tile([P, 1], f32)
            nc.sync.dma_start(b2_t[:], b2col[:])
            acc = cpool.tile([P, n_tiles, D_HID], f32)

            # ---- phase A: h' = xs @ W1 per piece; AG piece when stored
            B4 = 4
            for pnum in range(n_q):
                t0 = int(sum(piece_tiles[:pnum]))
                t1 = t0 + piece_tiles[pnum]
                for t4 in range(t0, t1, B4):
                    nb = min(B4, t1 - t4)
                    c0 = t4 * P
                    cn = min(sh, (t4 + nb) * P) - c0
                    xt = pool.tile([P, B4 * P], f32, tag="xt", bufs=3)
                    nc.sync.dma_start(xt[:, :cn], xT[:, c0 : c0 + cn])
                    hs4 = pool.tile([P, B4, D_HID], f32, tag="hs", bufs=3)
                    for j in range(nb):
                        t = t4 + j
                        pt = min(P, sh - t * P)
                        ph = psum_pool.tile(
                            [P, D_HID], f32, tag="ph", bufs=2, space="PSUM"
                        )
                        nc.tensor.matmul(
                            ph[:pt, :],
                            lhsT=xt[:, j * P : j * P + pt],
                            rhs=w1_t[:],
                            start=True,
                            stop=True,
                        )
                        nc.vector.tensor_copy(out=hs4[:pt, j, :], in_=ph[:pt, :])
                    prow = c0 - int(sum(pieces[:pnum]))
                    if cn == nb * P:
                        dst_ap = h_shp[pnum][prow : prow + cn, :].rearrange(
                            "(j p) d -> p j d", p=P
                        )
                        nc.sync.dma_start(dst_ap, hs4[:, :nb, :])
                    else:
                        for j in range(nb):
                            t = t4 + j
                            pt = min(P, sh - t * P)
                            pr = prow + j * P
                            nc.sync.dma_start(
                                h_shp[pnum][pr : pr + pt, :], hs4[:pt, j, :]
                            )
                nc.gpsimd.collective_compute(
                    "AllGather",
                    mybir.AluOpType.bypass,
                    replica_groups=rg,
                    ins=[h_shp[pnum][:]],
                    outs=[h_r[pnum][:]],
                )

            # ---- aggregation layers
            qn_state = [0]

            def do_layer(layer, table_r, self_p, out_p):
                pa_cur = {}  # t -> live psum tile for current (t, q) chain
                ag_fired = [False] * n_q

                def epilogue(s):
                    for t in sup_tiles[s]:
                        pt = min(P, sh - t * P)
                        pnum, prow = tile_piece[t]
                        st = pool.tile([P, D_HID], f32, tag=f"st{layer}", bufs=3)
                        if pt < P:
                            nc.vector.memset(st[:], 0.0)
                        nc.sync.dma_start(
                            st[:pt, :], self_p[pnum][prow : prow + pt, :]
                        )
                        dv = dinv_t[:pt, t : t + 1]
                        if first_q[t] >= 0:
                            u1 = pool.tile([P, D_HID], f32, tag=f"u1{layer}", bufs=3)
                            nc.vector.tensor_add(
                                u1[:pt, :], acc[:pt, t, :], st[:pt, :]
                            )
                        else:
                            u1 = st
                        if layer == 1:
                            t2 = pool.tile([P, D_HID], f32, tag="t2", bufs=3)
                            nc.vector.tensor_tensor(
                                out=t2[:pt, :],
                                in0=u1[:pt, :],
                                in1=dv.to_broadcast([pt, D_HID]),
                                op=mybir.AluOpType.mult,
                            )
                            t3 = pool.tile([P, D_HID], f32, tag="t3", bufs=3)
                            nc.vector.tensor_add(t3[:pt, :], t2[:pt, :], b1_t[:pt, :])
                            rr = pool.tile([P, D_HID], f32, tag="rr", bufs=3)
                            nc.scalar.activation(
                                rr[:pt, :],
                                t3[:pt, :],
                                mybir.ActivationFunctionType.Relu,
                            )
                            rp = pool.tile([P, D_HID], f32, tag="rp", bufs=3)
                            nc.vector.tensor_tensor(
                                out=rp[:pt, :],
                                in0=rr[:pt, :],
                                in1=dv.to_broadcast([pt, D_HID]),
                                op=mybir.AluOpType.mult,
                            )
                            nc.sync.dma_start(
                                out_p[pnum][prow : prow + pt, :], rp[:pt, :]
                            )
                        else:
                            u2 = pool.tile([P, D_HID], f32, tag="u2", bufs=3)
                            nc.vector.tensor_mul(u2[:pt, :], u1[:pt, :], w2_t[:pt, :])
                            yv = pool.tile([P, 1], f32, tag="yv", bufs=3)
                            nc.vector.tensor_reduce(
                                yv[:pt, :],
                                u2[:pt, :],
                                axis=mybir.AxisListType.X,
                                op=mybir.AluOpType.add,
                            )
                            ov = pool.tile([P, 1], f32, tag="ov", bufs=3)
                            nc.scalar.activation(
                                ov[:pt, :],
                                yv[:pt, :],
                                mybir.ActivationFunctionType.Sigmoid,
                                bias=b2_t[:pt, :],
                                scale=dv,
                            )
                            nc.sync.dma_start(y_out[t * P : t * P + pt, :], ov[:pt, :])

                for item in program:
                    if item[0] == "call":
                        c = calls[item[1]]
                        q, ni, nch = c["q"], c["ni"], c["nch"]
                        gbuf = pool.tile(
                            [P, MAXNCH, D_HID], f32, tag=f"g{layer}", bufs=3
                        )
                        nc.gpsimd.dma_gather(
                            gbuf[:, :nch, :],
                            table_r[q][0 : regions[q], :],
                            idx_t[:, c["o16"] : c["o16"] + ni // 16],
                            ni,
                            ni,
                            D_HID,
                            queue_num=qn_state[0] % 4,
                        )
                        qn_state[0] += 1
                        gbf = pool.tile(
                            [P, MAXNCH, D_HID], bf16, tag=f"gb{layer}", bufs=3
                        )
                        nc.scalar.copy(out=gbf[:, :nch, :], in_=gbuf[:, :nch, :])
                        oh = pool.tile([P, MAXNCH, P], bf16, tag=f"oh{layer}", bufs=3)
                        ch0 = c["chunk0"]
                        dls = dl_t[:, ch0 : ch0 + nch].rearrange(
                            "p (b o) -> p b o", o=1
                        )
                        nc.vector.tensor_tensor(
                            out=oh[:, :nch, :],
                            in0=dls.to_broadcast([P, nch, P]),
                            in1=iota_t[:, :nch, :],
                            op=mybir.AluOpType.is_equal,
                        )
                        for (k, a, b, t, st_f, sp_f) in c["segs"]:
                            if st_f:
                                pa_cur[t] = psum_pool.tile(
                                    [P, D_HID],
                                    f32,
                                    name=f"pa{t % 3}",
                                    tag=f"pa{t % 3}",
                                    bufs=2,
                                    space="PSUM",
                                )
                            nc.tensor.matmul(
                                pa_cur[t][:],
                                lhsT=oh[a:b, k, :],
                                rhs=gbf[a:b, k, :],
                                start=st_f,
                                stop=sp_f,
                            )
                            if sp_f:
                                # chain done: drain PSUM into the SBUF acc
                                if q == first_q[t]:
                                    nc.vector.tensor_copy(
                                        out=acc[:, t, :], in_=pa_cur[t][:]
                                    )
                                else:
                                    nc.vector.tensor_add(
                                        acc[:, t, :], acc[:, t, :], pa_cur[t][:]
                                    )
                    else:
                        _, s = item
                        epilogue(s)
                        if layer == 1:
                            done_t = sup_tiles[s][-1] + 1
                            for pnum in range(n_q):
                                if not ag_fired[pnum] and done_t >= cum_tiles[pnum]:
                                    nc.gpsimd.collective_compute(
                                        "AllGather",
                                        mybir.AluOpType.bypass,
                                        replica_groups=rg,
                                        ins=[out_p[pnum][:]],
                                        outs=[r_r[pnum][:]],
                                    )
                                    ag_fired[pnum] = True

            do_layer(1, h_r, h_shp, r_shp)
            do_layer(2, r_r, r_shp, None)

    nc.compile()
    return nc


# ---------------------------------------------------------------------------


def kernel(**inputs) -> np.ndarray:
    global LAST_RESULTS
    x = np.asarray(inputs["x"], dtype=np.float32)
    edge_index = np.asarray(inputs["edge_index"])
    w1_in = np.asarray(inputs["W1"], dtype=np.float32)
    b1_in = np.asarray(inputs["b1"], dtype=np.float32)
    w2_in = np.asarray(inputs["W2"], dtype=np.float32)
    b2_in = np.asarray(inputs["b2"], dtype=np.float32)

    n_nodes = x.shape[0]
    src = edge_index[0].astype(np.int64)
    dst = edge_index[1].astype(np.int64)

    deg = np.bincount(dst, minlength=n_nodes).astype(np.float64) + 1.0
    dinv = (1.0 / np.sqrt(deg)).astype(np.float32)

    meta, per_core = _build_plan(src, dst, n_nodes, N_CORES)
    sh = meta["sh"]
    n_tiles = meta["n_tiles"]

    nc = _build_program(meta)

    iota_arr = (
        np.broadcast_to(np.arange(P, dtype=np.float32), (P, MAXNCH, P))
        .astype(ml_dtypes.bfloat16)
        .copy()
    )
    b1bc = np.broadcast_to(b1_in.reshape(1, D_HID), (P, D_HID)).copy()
    w2bc = np.broadcast_to(w2_in.reshape(1, D_HID), (P, D_HID)).copy()

    xs = x * dinv[:, None]  # fold dinv into x (h' = (dinv.x) @ W1)

    in_maps = []
    for c in range(N_CORES):
        idx_arr, dst_arr = per_core[c]
        xT = np.ascontiguousarray(xs[c * sh : (c + 1) * sh].T)  # [128, sh]
        dv = np.zeros((P, n_tiles), dtype=np.float32)
        dsl = dinv[c * sh : (c + 1) * sh]
        for t in range(n_tiles):
            pt = min(P, sh - t * P)
            dv[:pt, t] = dsl[t * P : t * P + pt]
        in_maps.append(
            {
                "xT": xT,
                "w1": w1_in,
                "b1bc": b1bc,
                "w2bc": w2bc,
                "dinv_sh": dv,
                "iotab": iota_arr,
                "idx16": idx_arr,
                "dstloc": dst_arr.astype(ml_dtypes.bfloat16),
                "b2col": np.full((P, 1), float(b2_in.reshape(-1)[0]), dtype=np.float32),
            }
        )

    from concourse import bass_utils

    if os.environ.get("BASS_TRACE"):
        _install_axon_profile_shim()

    res = bass_utils.run_bass_kernel_spmd(
        nc,
        in_maps,
        core_ids=list(range(N_CORES)),
        trace=bool(os.environ.get("BASS_TRACE")),
        trace_cores=[0] if os.environ.get("BASS_TRACE") else None,
    )
    LAST_RESULTS = res
    out = np.concatenate([res.results[c]["y"] for c in range(N_CORES)], axis=0)
    return out.astype(np.float32)


# revision 11
# speedup vs baseline: 1.2252x; 1.1974x over previous
"""CreditRiskGNN (2-layer GCN) Trainium2 kernel, 8 NeuronCores — v2.

Sharding: nodes sharded across 8 cores; edges partitioned by destination so
scatter-adds are core-local; per-shard node features all-gathered per layer.

Math: GCNConv(x, W, b)[d] = dinv[d] * (sum_{e: dst=d} h'[src_e] + h'[d]) + b
with h' = dinv (.) (x @ W), dinv = rsqrt(indeg+1).  dinv is folded into x on
the host (h' = (dinv (.) x) @ W1), so phase A is a plain matmul.

v2 structure (one SPMD NEFF):
  A) h' per *piece* (shard split into 4 row-pieces), each piece AllGathered
     as soon as computed -> 4 pipelined AGs into 4 table regions in DRAM.
  B) Aggregation per layer is region(window)-major: pass q only needs AG
     piece q, so collectives hide behind gather/compute of earlier passes.
     Per-(tile, pass) partial sums accumulate in PSUM (bank-granular tiles,
     3 tags x 2 bufs) and drain into an SBUF accumulator per pass.
  C) Gather calls are packed per (super-tile x region) up to the 1024-index
     SWDGE cap (fewer ucode calls => less fixed Q7 overhead). Chunks that
     straddle tile boundaries are handled by partition-sliced matmul segments.
  D) One-hot dst-selection built per 128-chunk with DVE tensor_scalar
     is_equal (per-partition scalar => fast DVE mode, no broadcast penalty).
  E) Layer-1 epilogue per tile runs inside the last pass; r' pieces
     AllGather as soon as their tiles finish, so layer 2's table is ready
     the moment layer 1 ends.

Host does graph preprocessing only (edge sort/pad, gather-index layout,
node-relabeling for the table regions) and the final shard concat.
"""

import contextlib
import ctypes
import math
import os
import sys
import types

import ml_dtypes
import numpy as np

N_CORES = 8
P = 128
D_HID = 64
SUPER = 16                 # tiles per gather super-group
MAX_IDX = 1024             # HW descriptor-ring limit per dma_gather call
MAXNCH = MAX_IDX // P      # chunks per call

LAST_RESULTS = None  # BassKernelResults of the last run (for test harnesses)


# ---------------------------------------------------------------------------
# axon NTFF profile hook shim (only needed when BASS_TRACE=1 under axon)
def _install_axon_profile_shim():
    if "antenv.axon_hooks" in sys.modules:
        return
    try:
        so_path = "/opt/axon/libaxon_pjrt.so"
        if not os.path.exists(so_path):
            return
        lib = ctypes.CDLL(so_path)
        if not hasattr(lib, "axon_start_nrt_profile"):
            return
        lib.axon_start_nrt_profile.argtypes = [
            ctypes.POINTER(ctypes.c_int64),
            ctypes.c_size_t,
        ]
        lib.axon_start_nrt_profile.restype = ctypes.c_int64
        lib.axon_stop_nrt_profile.argtypes = [ctypes.c_char_p]
        lib.axon_stop_nrt_profile.restype = ctypes.c_int64

        @contextlib.contextmanager
        def _hook(output_dir, device_ids):
            import jax

            jax.devices()
            if device_ids:
                ids = (ctypes.c_int64 * len(device_ids))(*device_ids)
                rc = lib.axon_start_nrt_profile(ids, len(device_ids))
            else:
                rc = lib.axon_start_nrt_profile(None, 0)
            if rc != 0:
                raise RuntimeError(f"axon_start_nrt_profile rc={rc}")
            try:
                yield
            finally:
                n = lib.axon_stop_nrt_profile(str(output_dir).encode())
                if n < 0:
                    raise RuntimeError(f"axon_stop_nrt_profile rc={n}")

        mod = types.ModuleType("antenv.axon_hooks")
        _state = {"hook": _hook}
        mod.set_axon_ntff_profile_hook = lambda h: _state.__setitem__("hook", h)
        mod.get_axon_ntff_profile_hook = lambda: _state["hook"]
        sys.modules["antenv.axon_hooks"] = mod
        import antenv

        antenv.axon_hooks = mod
    except Exception:
        pass


# ---------------------------------------------------------------------------
# Host-side graph preprocessing


def _build_plan(src, dst, n_nodes, n_cores):
    """Shared (cross-core-uniform) program structure + per-core gather data.

    Table layout: 4 regions; region q holds rows [c*pieces[q] + r] for shard
    row r in piece q of core c (so AllGather piece q fills region q exactly).
    """
    sh = n_nodes // n_cores
    n_tiles = math.ceil(sh / P)
    piece = math.ceil(sh / 4 / P) * P
    pieces = [piece, piece, piece, sh - 3 * piece]
    assert pieces[3] > 0
    piece_starts = np.array([0, piece, 2 * piece, 3 * piece], dtype=np.int64)
    regions = [n_cores * pc for pc in pieces]
    n_q = 4
    n_sup = math.ceil(n_tiles / SUPER)
    sup_tiles = [
        list(range(s * SUPER, min((s + 1) * SUPER, n_tiles))) for s in range(n_sup)
    ]

    # --- per-core edge partition, sorted by (tile, region, table-idx)
    core_of = dst // sh
    counts = np.zeros((n_cores, n_tiles, n_q), dtype=np.int64)
    per_core_sorted = []
    for c in range(n_cores):
        m = core_of == c
        s_c = src[m].astype(np.int64)
        d_c = (dst[m] - c * sh).astype(np.int64)
        c_s, r_s = np.divmod(s_c, sh)
        q_s = np.searchsorted(piece_starts, r_s, side="right") - 1
        tab = c_s * np.array(pieces)[q_s] + (r_s - piece_starts[q_s])
        t_of = d_c // P
        key = t_of * n_q + q_s
        order = np.lexsort((tab, key))
        s_key = key[order]
        tab_s, d_s = tab[order], d_c[order]
        allk = np.arange(n_tiles * n_q)
        starts = np.searchsorted(s_key, allk, side="left").reshape(n_tiles, n_q)
        ends = np.searchsorted(s_key, allk, side="right").reshape(n_tiles, n_q)
        counts[c] = ends - starts
        per_core_sorted.append((tab_s, d_s, starts))

    # pad each (tile, region) run to a multiple of 16 (idx-wrap granularity).
    # One gather call per run: ~550-idx calls sit at the SWDGE sweet spot —
    # bigger calls stall the Q7 ucode on descriptor-ring space.
    mx = counts.max(axis=0)
    padded = ((mx + 15) // 16) * 16  # [n_tiles, n_q]; 0 stays 0

    # first pass (q) with edges, per tile — drain uses copy there, add after
    first_q = np.full(n_tiles, -1, dtype=np.int64)
    for t in range(n_tiles):
        nz = np.nonzero(padded[t])[0]
        if len(nz):
            first_q[t] = nz[0]

    # --- processing order: (q, s) groups; calls packed to MAX_IDX
    calls = []  # dict: q, o16, ni, nch, chunk0, segs
    slot_t_parts, slot_q_parts, slot_rank_parts = [], [], []
    call_starts = []
    total_idx = 0
    total_chunks = 0
    program = []  # ('call', ci) | ('drain', q, s) | ('epi', s)

    for q in range(n_q):
        for s in range(n_sup):
            runs = [(t, int(padded[t, q])) for t in sup_tiles[s] if padded[t, q] > 0]
            ni_group = sum(n for _, n in runs)
            if ni_group == 0:
                if q == n_q - 1:
                    program.append(("epi", s))
                continue
            g_t = np.concatenate([np.full(n, t, dtype=np.int64) for t, n in runs])
            g_rank = np.concatenate([np.arange(n, dtype=np.int64) for _, n in runs])
            slot_t_parts.append(g_t)
            slot_q_parts.append(np.full(ni_group, q, dtype=np.int64))
            slot_rank_parts.append(g_rank)
            # one gather call per (tile, region) run
            done = 0
            for t, ni in runs:
                ci = len(calls)
                call_starts.append(total_idx + done)
                nch = (ni + P - 1) // P
                segs = []
                for k in range(nch):
                    a = k * P
                    b = min((k + 1) * P, ni)
                    segs.append([k, 0, b - a, t, k == 0, k == nch - 1])
                calls.append(
                    dict(q=q, o16=(total_idx + done) // 16, ni=ni, nch=nch,
                         chunk0=total_chunks, segs=segs)
                )
                total_chunks += nch
                program.append(("call", ci))
                done += ni
            if q == n_q - 1:
                program.append(("epi", s))
            total_idx += ni_group

    slot_t = np.concatenate(slot_t_parts)
    slot_q = np.concatenate(slot_q_parts)
    slot_rank = np.concatenate(slot_rank_parts)
    call_starts_a = np.array(call_starts, dtype=np.int64)

    # global slot -> (chunk col, chunk row) for dstloc
    i_all = np.arange(total_idx, dtype=np.int64)
    ci_of = np.searchsorted(call_starts_a, i_all, side="right") - 1
    rel = i_all - call_starts_a[ci_of]
    chunk0_of = np.array([c["chunk0"] for c in calls], dtype=np.int64)[ci_of]
    col_of = chunk0_of + rel // P
    row_of = rel % P

    meta = dict(
        n_nodes=n_nodes,
        sh=sh,
        n_tiles=n_tiles,
        pieces=pieces,
        regions=regions,
        n_q=n_q,
        sup_tiles=sup_tiles,
        calls=calls,
        program=program,
        total_idx=total_idx,
        total_chunks=total_chunks,
        padded=padded,
        first_q=first_q,
    )

    # --- per-core gather index / dstloc data
    per_core = []
    for c in range(n_cores):
        tab_s, d_s, starts = per_core_sorted[c]
        cnt = counts[c][slot_t, slot_q]
        st = starts[slot_t, slot_q]
        valid = slot_rank < cnt
        if len(tab_s):
            pos = np.clip(st + slot_rank, 0, len(tab_s) - 1)
            idxv = np.where(valid, tab_s[pos], 0).astype(np.int16)
            dstv = np.where(valid, d_s[pos] % P, -1).astype(np.float32)
        else:
            idxv = np.zeros(total_idx, dtype=np.int16)
            dstv = np.full(total_idx, -1.0, dtype=np.float32)
        arr16 = np.zeros((16, total_idx // 16), dtype=np.int16)
        arr16[i_all % 16, i_all // 16] = idxv
        idx_arr = np.tile(arr16, (8, 1))
        dst_arr = np.full((P, total_chunks), -1.0, dtype=np.float32)
        dst_arr[row_of, col_of] = dstv
        per_core.append((idx_arr, dst_arr))
    return meta, per_core


# ---------------------------------------------------------------------------
# Device program


def _build_program(meta):
    import concourse.bacc as bacc
    import concourse.mybir as mybir
    import concourse.tile as tile

    sh = meta["sh"]
    n_tiles = meta["n_tiles"]
    pieces = meta["pieces"]
    regions = meta["regions"]
    n_q = meta["n_q"]
    sup_tiles = meta["sup_tiles"]
    calls = meta["calls"]
    program = meta["program"]
    total_idx = meta["total_idx"]
    total_chunks = meta["total_chunks"]
    padded = meta["padded"]
    first_q = meta["first_q"]

    piece_tiles = [(pc + P - 1) // P for pc in pieces]
    # tile -> (piece, row offset within piece)
    tile_piece = []
    for t in range(n_tiles):
        row = t * P
        pacc = 0
        for p in range(n_q):
            if row < pacc + pieces[p]:
                tile_piece.append((p, row - pacc))
                break
            pacc += pieces[p]
    # piece p fully epilogued once tiles < cum_tiles[p] are done
    cum_tiles = np.cumsum(piece_tiles)

    f32 = mybir.dt.float32
    bf16 = mybir.dt.bfloat16
    nc = bacc.Bacc("TRN2", target_bir_lowering=False, debug=False, num_swdge_queues=4)

    xT = nc.dram_tensor("xT", [P, sh], f32, kind="ExternalInput")
    w1 = nc.dram_tensor("w1", [P, D_HID], f32, kind="ExternalInput")
    b1bc = nc.dram_tensor("b1bc", [P, D_HID], f32, kind="ExternalInput")
    w2bc = nc.dram_tensor("w2bc", [P, D_HID], f32, kind="ExternalInput")
    dinv_sh = nc.dram_tensor("dinv_sh", [P, n_tiles], f32, kind="ExternalInput")
    iotab = nc.dram_tensor("iotab", [P, MAXNCH, P], bf16, kind="ExternalInput")
    idx16 = nc.dram_tensor(
        "idx16", [P, total_idx // 16], mybir.dt.int16, kind="ExternalInput"
    )
    dstloc = nc.dram_tensor("dstloc", [P, total_chunks], bf16, kind="ExternalInput")
    b2col = nc.dram_tensor("b2col", [P, 1], f32, kind="ExternalInput")
    y_out = nc.dram_tensor("y", [sh, 1], f32, kind="ExternalOutput")

    h_shp = [
        nc.dram_tensor(f"h_sh{p}", [pieces[p], D_HID], f32, kind="Internal")
        for p in range(n_q)
    ]
    h_r = [
        nc.dram_tensor(
            f"h_r{p}", [regions[p], D_HID], f32, kind="Internal", addr_space="Shared"
        )
        for p in range(n_q)
    ]
    r_shp = [
        nc.dram_tensor(f"r_sh{p}", [pieces[p], D_HID], f32, kind="Internal")
        for p in range(n_q)
    ]
    r_r = [
        nc.dram_tensor(
            f"r_r{p}", [regions[p], D_HID], f32, kind="Internal", addr_space="Shared"
        )
        for p in range(n_q)
    ]

    rg = [list(range(N_CORES))]

    with tile.TileContext(nc) as tc:
        with (
            tc.tile_pool(name="const", bufs=1) as cpool,
            tc.tile_pool(name="sbuf", bufs=1) as pool,
            tc.tile_pool(name="psum", bufs=1, space="PSUM") as psum_pool,
        ):
            w1_t = cpool.tile([P, D_HID], f32)
            nc.sync.dma_start(w1_t[:], w1[:])
            b1_t = cpool.tile([P, D_HID], f32)
            nc.sync.dma_start(b1_t[:], b1bc[:])
            w2_t = cpool.tile([P, D_HID], f32)
            nc.sync.dma_start(w2_t[:], w2bc[:])
            dinv_t = cpool.tile([P, n_tiles], f32)
            nc.sync.dma_start(dinv_t[:], dinv_sh[:])
            iota_t = cpool.tile([P, MAXNCH, P], bf16)
            nc.sync.dma_start(iota_t[:], iotab[:])
            idx_t = cpool.tile([P, total_idx // 16], mybir.dt.int16)
            nc.sync.dma_start(idx_t[:], idx16[:])
            dl_t = cpool.tile([P, total_chunks], bf16)
            nc.sync.dma_start(dl_t[:], dstloc[:])
            b2_t = cpool.tile([P, 1], f32)
            nc.sync.dma_start(b2_t[:], b2col[:])
            acc = cpool.tile([P, n_tiles, D_HID], f32)

            # ---- phase A: h' = xs @ W1 per piece; AG piece when stored
            B4 = 4
            for pnum in range(n_q):
                t0 = int(sum(piece_tiles[:pnum]))
                t1 = t0 + piece_tiles[pnum]
                for t4 in range(t0, t1, B4):
                    nb = min(B4, t1 - t4)
                    c0 = t4 * P
                    cn = min(sh, (t4 + nb) * P) - c0
                    xt = pool.tile([P, B4 * P], f32, tag="xt", bufs=3)
                    nc.sync.dma_start(xt[:, :cn], xT[:, c0 : c0 + cn])
                    hs4 = pool.tile([P, B4, D_HID], f32, tag="hs", bufs=3)
                    for j in range(nb):
                        t = t4 + j
                        pt = min(P, sh - t * P)
                        ph = psum_pool.tile(
                            [P, D_HID], f32, tag="ph", bufs=2, space="PSUM"
                        )
                        nc.tensor.matmul(
                            ph[:pt, :],
                            lhsT=xt[:, j * P : j * P + pt],
                            rhs=w1_t[:],
                            start=True,
                            stop=True,
                        )
                        nc.vector.tensor_copy(out=hs4[:pt, j, :], in_=ph[:pt, :])
                    prow = c0 - int(sum(pieces[:pnum]))
                    if cn == nb * P:
                        dst_ap = h_shp[pnum][prow : prow + cn, :].rearrange(
                            "(j p) d -> p j d", p=P
                        )
                        nc.sync.dma_start(dst_ap, hs4[:, :nb, :])
                    else:
                        for j in range(nb):
                            t = t4 + j
                            pt = min(P, sh - t * P)
                            pr = prow + j * P
                            nc.sync.dma_start(
                                h_shp[pnum][pr : pr + pt, :], hs4[:pt, j, :]
                            )
                nc.gpsimd.collective_compute(
                    "AllGather",
                    mybir.AluOpType.bypass,
                    replica_groups=rg,
                    ins=[h_shp[pnum][:]],
                    outs=[h_r[pnum][:]],
                )

            # ---- aggregation layers
            qn_state = [0]

            def do_layer(layer, table_r, self_p, out_p):
                pa_cur = {}  # t -> live psum tile for current (t, q) chain
                ag_fired = [False] * n_q

                def epilogue(s):
                    for t in sup_tiles[s]:
                        pt = min(P, sh - t * P)
                        pnum, prow = tile_piece[t]
                        st = pool.tile([P, D_HID], f32, tag=f"st{layer}", bufs=3)
                        if pt < P:
                            nc.vector.memset(st[:], 0.0)
                        nc.sync.dma_start(
                            st[:pt, :], self_p[pnum][prow : prow + pt, :]
                        )
                        dv = dinv_t[:pt, t : t + 1]
                        if first_q[t] >= 0:
                            u1 = pool.tile([P, D_HID], f32, tag=f"u1{layer}", bufs=3)
                            nc.vector.tensor_add(
                                u1[:pt, :], acc[:pt, t, :], st[:pt, :]
                            )
                        else:
                            u1 = st
                        if layer == 1:
                            t2 = pool.tile([P, D_HID], f32, tag="t2", bufs=3)
                            nc.vector.tensor_tensor(
                                out=t2[:pt, :],
                                in0=u1[:pt, :],
                                in1=dv.to_broadcast([pt, D_HID]),
                                op=mybir.AluOpType.mult,
                            )
                            t3 = pool.tile([P, D_HID], f32, tag="t3", bufs=3)
                            nc.vector.tensor_add(t3[:pt, :], t2[:pt, :], b1_t[:pt, :])
                            rr = pool.tile([P, D_HID], f32, tag="rr", bufs=3)
                            nc.scalar.activation(
                                rr[:pt, :],
                                t3[:pt, :],
                                mybir.ActivationFunctionType.Relu,
                            )
                            rp = pool.tile([P, D_HID], f32, tag="rp", bufs=3)
                            nc.vector.tensor_tensor(
                                out=rp[:pt, :],
                                in0=rr[:pt, :],
                                in1=dv.to_broadcast([pt, D_HID]),
                                op=mybir.AluOpType.mult,
                            )
                            nc.sync.dma_start(
                                out_p[pnum][prow : prow + pt, :], rp[:pt, :]
                            )
                        else:
                            u2 = pool.tile([P, D_HID], f32, tag="u2", bufs=3)
                            nc.vector.tensor_mul(u2[:pt, :], u1[:pt, :], w2_t[:pt, :])
                            yv = pool.tile([P, 1], f32, tag="yv", bufs=3)
                            nc.vector.tensor_reduce(
                                yv[:pt, :],
                                u2[:pt, :],
                                axis=mybir.AxisListType.X,
                                op=mybir.AluOpType.add,
                            )
                            ov = pool.tile([P, 1], f32, tag="ov", bufs=3)
                            nc.scalar.activation(
                                ov[:pt, :],
                                yv[:pt, :],
                                mybir.ActivationFunctionType.Sigmoid,
                                bias=b2_t[:pt, :],
                                scale=dv,
                            )
                            nc.sync.dma_start(y_out[t * P : t * P + pt, :], ov[:pt, :])

                for item in program:
                    if item[0] == "call":
                        c = calls[item[1]]
                        q, ni, nch = c["q"], c["ni"], c["nch"]
                        gbuf = pool.tile(
                            [P, MAXNCH, D_HID], f32, tag=f"g{layer}", bufs=3
                        )
                        nc.gpsimd.dma_gather(
                            gbuf[:, :nch, :],
                            table_r[q][0 : regions[q], :],
                            idx_t[:, c["o16"] : c["o16"] + ni // 16],
                            ni,
                            ni,
                            D_HID,
                            queue_num=qn_state[0] % 4,
                        )
                        qn_state[0] += 1
                        gbf = pool.tile(
                            [P, MAXNCH, D_HID], bf16, tag=f"gb{layer}", bufs=3
                        )
                        nc.scalar.copy(out=gbf[:, :nch, :], in_=gbuf[:, :nch, :])
                        oh = pool.tile([P, MAXNCH, P], bf16, tag=f"oh{layer}", bufs=3)
                        ch0 = c["chunk0"]
                        dls = dl_t[:, ch0 : ch0 + nch].rearrange(
                            "p (b o) -> p b o", o=1
                        )
                        nc.vector.tensor_tensor(
                            out=oh[:, :nch, :],
                            in0=dls.to_broadcast([P, nch, P]),
                            in1=iota_t[:, :nch, :],
                            op=mybir.AluOpType.is_equal,
                        )
                        for (k, a, b, t, st_f, sp_f) in c["segs"]:
                            if st_f:
                                pa_cur[t] = psum_pool.tile(
                                    [P, D_HID],
                                    f32,
                                    name=f"pa{t % 3}",
                                    tag=f"pa{t % 3}",
                                    bufs=2,
                                    space="PSUM",
                                )
                            nc.tensor.matmul(
                                pa_cur[t][:],
                                lhsT=oh[a:b, k, :],
                                rhs=gbf[a:b, k, :],
                                start=st_f,
                                stop=sp_f,
                            )
                            if sp_f:
                                # chain done: drain PSUM into the SBUF acc
                                if q == first_q[t]:
                                    nc.vector.tensor_copy(
                                        out=acc[:, t, :], in_=pa_cur[t][:]
                                    )
                                else:
                                    nc.vector.tensor_add(
                                        acc[:, t, :], acc[:, t, :], pa_cur[t][:]
                                    )
                    else:
                        _, s = item
                        epilogue(s)
                        if layer == 1:
                            done_t = sup_tiles[s][-1] + 1
                            for pnum in range(n_q):
                                if not ag_fired[pnum] and done_t >= cum_tiles[pnum]:
                                    nc.gpsimd.collective_compute(
                                        "AllGather",
                                        mybir.AluOpType.bypass,
                                        replica_groups=rg,
                                        ins=[out_p[pnum][:]],
                                        outs=[r_r[pnum][:]],
                                    )
                                    ag_fired[pnum] = True

            do_layer(1, h_r, h_shp, r_shp)
            do_layer(2, r_r, r_shp, None)

    nc.compile()
    return nc


# ---------------------------------------------------------------------------


def kernel(**inputs) -> np.ndarray:
    global LAST_RESULTS
    x = np.asarray(inputs["x"], dtype=np.float32)
    edge_index = np.asarray(inputs["edge_index"])
    w1_in = np.asarray(inputs["W1"], dtype=np.float32)
    b1_in = np.asarray(inputs["b1"], dtype=np.float32)
    w2_in = np.asarray(inputs["W2"], dtype=np.float32)
    b2_in = np.asarray(inputs["b2"], dtype=np.float32)

    n_nodes = x.shape[0]
    src = edge_index[0].astype(np.int64)
    dst = edge_index[1].astype(np.int64)

    deg = np.bincount(dst, minlength=n_nodes).astype(np.float64) + 1.0
    dinv = (1.0 / np.sqrt(deg)).astype(np.float32)

    meta, per_core = _build_plan(src, dst, n_nodes, N_CORES)
    sh = meta["sh"]
    n_tiles = meta["n_tiles"]

    nc = _build_program(meta)

    iota_arr = (
        np.broadcast_to(np.arange(P, dtype=np.float32), (P, MAXNCH, P))
        .astype(ml_dtypes.bfloat16)
        .copy()
    )
    b1bc = np.broadcast_to(b1_in.reshape(1, D_HID), (P, D_HID)).copy()
    w2bc = np.broadcast_to(w2_in.reshape(1, D_HID), (P, D_HID)).copy()

    xs = x * dinv[:, None]  # fold dinv into x (h' = (dinv.x) @ W1)

    in_maps = []
    for c in range(N_CORES):
        idx_arr, dst_arr = per_core[c]
        xT = np.ascontiguousarray(xs[c * sh : (c + 1) * sh].T)  # [128, sh]
        dv = np.zeros((P, n_tiles), dtype=np.float32)
        dsl = dinv[c * sh : (c + 1) * sh]
        for t in range(n_tiles):
            pt = min(P, sh - t * P)
            dv[:pt, t] = dsl[t * P : t * P + pt]
        in_maps.append(
            {
                "xT": xT,
                "w1": w1_in,
                "b1bc": b1bc,
                "w2bc": w2bc,
                "dinv_sh": dv,
                "iotab": iota_arr,
                "idx16": idx_arr,
                "dstloc": dst_arr.astype(ml_dtypes.bfloat16),
                "b2col": np.full((P, 1), float(b2_in.reshape(-1)[0]), dtype=np.float32),
            }
        )

    from concourse import bass_utils

    if os.environ.get("BASS_TRACE"):
        _install_axon_profile_shim()

    res = bass_utils.run_bass_kernel_spmd(
        nc,
        in_maps,
        core_ids=list(range(N_CORES)),
        trace=bool(os.environ.get("BASS_TRACE")),
        trace_cores=[0] if os.environ.get("BASS_TRACE") else None,
    )
    LAST_RESULTS = res
    out = np.concatenate([res.results[c]["y"] for c in range(N_CORES)], axis=0)
    return out.astype(np.float32)
